# revision 1
# baseline (speedup 1.0000x reference)
"""Adjacency-aware multi-head attention on 8 trn2 NeuronCores.

Math (per b, head k):
  Q = h[b] @ Wq[:, k] + bq[k]           [N, D]
  S[i, j] = (Q_i . K_j) / sqrt(D)
  P[j, i] = exp(S[i, j]) / sum_j exp(S[i, j])      (softmax over keys j)
  out[i, d] = sum_j P[j, i] * A[b, j, i] * V[j, d]

Sharding: 16 (b, head) pairs over 8 cores, 2 heads of the SAME b per core so
the A[b] stream is shared by both heads.

Device dataflow ([j, i] "transposed" layout so A needs no transpose):
  S^T[j-tile, i-chunk] on PE: K^T tile stationary (bf16), Q^T moving (bf16).
    K^T is packed so j-tile t lives on partition strip 32*(t%4) and Q^T is
    replicated to all 4 strips -> 4 j-tiles run CONCURRENTLY on the PE's
    four 32-row groups.
  exp on ACT: PSUM -> SBUF bf16 (groups of 3 PSUM banks per op)
  EA = E * A on DVE (bf16 tensor_tensor, 2x mode; A broadcast over heads)
  Phase 2 (lagged one chunk behind S so the in-order PE queue never stalls),
  4 concurrent col-group accumulation streams into one PSUM tile:
    cols  0-31: outT_h0 += V_h0[j]^T @ EA_h0     (M=32)
    cols 32-63: outT_h1 += V_h1[j]^T @ EA_h1     (M=32)
    col  64: denom_h0 += ones^T @ E_h0           (M=1)
    col  96: denom_h1 += ones^T @ E_h1           (M=1)
Device returns [128, N]: rows 0-31 outT_h0, 32-63 outT_h1, rows 64/96 the
softmax denominators.  Host does out = (outT / denom)^T plus the gather.
"""

import math
import os

import numpy as np
import ml_dtypes

B, N, IN_DIM = 2, 2048, 256
HEADS, D = 8, 32
NCORES = 8
HPC = 2              # heads per core
NJ = N // 128        # 16 j-tiles
NCH = 4              # i-chunks
CH = N // NCH        # 512
CORES_PER_B = NCORES // B

LAST_RESULTS = None  # BassKernelResults of the most recent kernel() call


def _build_bass():
    import concourse.bass as bass
    import concourse.mybir as mybir
    import concourse.tile as tile
    from concourse import bacc

    f32 = mybir.dt.float32
    bf16 = mybir.dt.bfloat16
    AF = mybir.ActivationFunctionType

    nc = bacc.Bacc("TRN2", target_bir_lowering=False, debug=False,
                   num_devices=NCORES)

    hT = nc.dram_tensor("hT", [IN_DIM, N], bf16, kind="ExternalInput").ap()
    Ab = nc.dram_tensor("Ab", [N, N], bf16, kind="ExternalInput").ap()
    wq = nc.dram_tensor("wq", [IN_DIM, HPC * D], bf16, kind="ExternalInput").ap()
    wk = nc.dram_tensor("wk", [IN_DIM, HPC * D], bf16, kind="ExternalInput").ap()
    wv = nc.dram_tensor("wv", [IN_DIM, HPC * D], bf16, kind="ExternalInput").ap()
    bq = nc.dram_tensor("bq", [HPC * D, 1], f32, kind="ExternalInput").ap()
    bk4 = nc.dram_tensor("bk4", [128, HPC], f32, kind="ExternalInput").ap()
    bvb = nc.dram_tensor("bvb", [128, HPC * D], f32, kind="ExternalInput").ap()
    o = nc.dram_tensor("o", [128, N], f32, kind="ExternalOutput").ap()

    SC = 1.0 / math.sqrt(D)

    with (
        tile.TileContext(nc) as tc,
        tc.tile_pool(name="const", bufs=1) as cpool,
        tc.tile_pool(name="ps", bufs=2, space="PSUM") as pspool,
        tc.tile_pool(name="pod", bufs=2, space="PSUM") as podpool,
        tc.tile_pool(name="apool", bufs=2) as apool,
        tc.tile_pool(name="epool", bufs=2) as epool,
        tc.tile_pool(name="eapool", bufs=2) as eapool,
        tc.tile_pool(name="opool", bufs=3) as opool,
    ):
        # ---- constants / inputs into SBUF
        hT_sb = cpool.tile([128, 2, N], bf16, tag="hT")
        nc.sync.dma_start(hT_sb, hT.rearrange("(s p) n -> p s n", p=128))
        w_sb = {}
        for name, ap in (("q", wq), ("k", wk), ("v", wv)):
            t = cpool.tile([128, 2, HPC * D], bf16, tag=f"w{name}")
            nc.sync.dma_start(t, ap.rearrange("(s p) m -> p s m", p=128))
            w_sb[name] = t
        bq_sb = cpool.tile([HPC * D, 1], f32, tag="bq")
        nc.sync.dma_start(bq_sb, bq)
        bk4_sb = cpool.tile([128, HPC], f32, tag="bk4")
        nc.sync.dma_start(bk4_sb, bk4)
        bvb_sb = cpool.tile([128, HPC * D], f32, tag="bvb")
        nc.sync.dma_start(bvb_sb, bvb)
        ones_sb = cpool.tile([128, 1], bf16, tag="ones")
        nc.vector.memset(ones_sb, 1.0)


        # Q^T replicated on all 4 partition strips; K^T packed so j-tile t
        # sits on strip 32*(t%4), column block t//4.
        qt4 = [cpool.tile([128, N], bf16, tag=f"qt4{h}", name=f"qt4{h}")
               for h in range(HPC)]
        kt4 = [cpool.tile([128, NJ // 4, 128], bf16, tag=f"kt4{h}",
                          name=f"kt4{h}") for h in range(HPC)]
        Vt = cpool.tile([128, NJ, HPC * D], bf16, tag="vt")

        def bcast_free(ap_col, n):
            return bass.AP(tensor=ap_col.tensor, offset=ap_col.offset,
                           ap=[ap_col.ap[0], [0, n]])

        # ---- K projection straight into the packed kt4 layout: for strip r
        #      the moving operand picks j-tiles {r, r+4, r+8, r+12}
        for h in range(HPC):
            ps = pspool.tile([128, 4 * 128], f32, tag="ps")
            for r in range(4):
                for s in range(2):
                    base = hT_sb[:, s, r * 128:(r + 1) * 128]
                    rhs = bass.AP(
                        tensor=base.tensor, offset=base.offset,
                        ap=[base.ap[0], [4 * 128, 4], [1, 128]],
                    )
                    nc.tensor.matmul(
                        ps[32 * r:32 * r + 32, :],
                        lhsT=w_sb["k"][:, s, h * D:(h + 1) * D],
                        rhs=rhs,
                        start=(s == 0), stop=(s == 1),
                        tile_position=(0, 32 * r),
                    )
            nc.vector.scalar_tensor_tensor(
                kt4[h].rearrange("p q jj -> p (q jj)"), ps, 1.0,
                bcast_free(bk4_sb[:, h:h + 1], 4 * 128),
                op0=mybir.AluOpType.mult, op1=mybir.AluOpType.add,
            )

        # ---- Q projection (+bias, scaled 1/sqrt(D)) per quarter, each
        #      strip-replicated immediately; chunk ch only needs quarter ch
        bq_bcast = bcast_free(bq_sb, CH)

        def emit_qproj(quarter):
            sl = slice(quarter * CH, (quarter + 1) * CH)
            ps = pspool.tile([HPC * D, CH], f32, tag="ps", name="qps")
            for s in range(2):
                nc.tensor.matmul(
                    ps, lhsT=w_sb["q"][:, s, :], rhs=hT_sb[:, s, sl],
                    start=(s == 0), stop=(s == 1),
                )
            nc.vector.scalar_tensor_tensor(
                qt4[0][0:D, sl], ps[0:D, :], SC, bq_bcast[0:D, :],
                op0=mybir.AluOpType.mult, op1=mybir.AluOpType.add,
            )
            nc.vector.scalar_tensor_tensor(
                qt4[1][D:2 * D, sl], ps[D:2 * D, :], SC, bq_bcast[D:2 * D, :],
                op0=mybir.AluOpType.mult, op1=mybir.AluOpType.add,
            )
            for r in range(1, 4):
                nc.sync.dma_start(qt4[0][32 * r:32 * r + 32, sl],
                                  qt4[0][0:D, sl])
            for r in (0, 2, 3):
                nc.sync.dma_start(qt4[1][32 * r:32 * r + 32, sl],
                                  qt4[1][D:2 * D, sl])

        for quarter in range(4):
            emit_qproj(quarter)

        # ---- projection V[j, d] for both heads (+bias via broadcast tile)
        for t in range(NJ):
            ps = pspool.tile([128, HPC * D], f32, tag="ps")
            for s in range(2):
                nc.tensor.matmul(
                    ps, lhsT=hT_sb[:, s, t * 128:(t + 1) * 128],
                    rhs=w_sb["v"][:, s, :],
                    start=(s == 0), stop=(s == 1),
                )
            nc.vector.tensor_add(Vt[:, t, :], ps, bvb_sb)

        # ---- main loop (phase2 lags one chunk so the in-order PE queue
        #      never stalls waiting on exp/A-mult of the same chunk)
        A3 = Ab.rearrange("(t p) i -> p t i", p=128)

        # e/ea column block for (head hh, j-tile t):
        def blk(hh, t):
            return (t // 4) * 8 + hh * 4 + (t % 4)

        def emit_phase2_tile(od, e_t, ea_t, t):
            first, last = (t == 0), (t == NJ - 1)
            sh0 = slice(blk(0, t) * CH, (blk(0, t) + 1) * CH)
            sh1 = slice(blk(1, t) * CH, (blk(1, t) + 1) * CH)
            nc.tensor.matmul(
                od[0:D, :], lhsT=Vt[:, t, 0:D], rhs=ea_t[:, sh0],
                start=first, stop=last, tile_position=(0, 0),
            )
            nc.tensor.matmul(
                od[D:2 * D, :], lhsT=Vt[:, t, D:2 * D], rhs=ea_t[:, sh1],
                start=first, stop=last, tile_position=(0, 32),
            )
            nc.tensor.matmul(
                od[64:65, :], lhsT=ones_sb, rhs=e_t[:, sh0],
                start=first, stop=last, tile_position=(0, 64),
            )
            nc.tensor.matmul(
                od[96:97, :], lhsT=ones_sb, rhs=e_t[:, sh1],
                start=first, stop=last, tile_position=(0, 96),
            )

        def emit_phase2(e_t, ea_t, ch):
            od = podpool.tile([128, CH], f32, tag="od")
            for t in range(NJ):
                emit_phase2_tile(od, e_t, ea_t, t)
            o_sb = opool.tile([128, CH], f32, tag="o")
            nc.vector.tensor_copy(o_sb, od)
            nc.sync.dma_start(o[:, ch * CH:(ch + 1) * CH], o_sb)

        GRP = 3              # S psum banks per exp op
        nblocks = NJ * HPC
        pending = None
        for ch in range(NCH):
            a_t = apool.tile([128, NJ * CH], bf16, tag="a")
            nc.sync.dma_start(
                a_t.rearrange("p (t i) -> p t i", i=CH),
                A3[:, :, ch * CH:(ch + 1) * CH],
            )
            e_t = epool.tile([128, nblocks * CH], bf16, tag="e")
            ea_t = eapool.tile([128, nblocks * CH], bf16, tag="ea")
            # S matmuls: block b = q*8 + hh*4 + r covers j-tile t = 4q + r;
            # runs of 4 same-head blocks occupy the 4 PE row-groups and run
            # concurrently.  exp drains GRP psum banks per op.  phase2 MMs
            # of the previous chunk are interleaved to fill exp-gated PE
            # stalls.
            def emit_sblocks(b_lo, b_hi):
                ps = None
                gsz = 0
                for b in range(b_lo, b_hi):
                    q, hh, r = b // 8, (b // 4) % 2, b % 4
                    g = (b - b_lo) % GRP
                    if g == 0:
                        gsz = min(GRP, b_hi - b)
                        ps = pspool.tile([128, GRP * CH], f32, tag="ps",
                                         name="sps")
                    nc.tensor.matmul(
                        ps[:, g * CH:(g + 1) * CH],
                        lhsT=kt4[hh][32 * r:32 * r + 32, q, :],
                        rhs=qt4[hh][32 * r:32 * r + 32,
                                    ch * CH:(ch + 1) * CH],
                        start=True, stop=True,
                        tile_position=(32 * r, 0),
                    )
                    if g == gsz - 1:
                        b0 = b - g
                        nc.scalar.activation(
                            e_t[:, b0 * CH:(b + 1) * CH], ps[:, :gsz * CH],
                            AF.Exp)

            emit_sblocks(0, nblocks)
            for t in range(NJ):
                b0 = blk(0, t)
                base = e_t[:, b0 * CH:(b0 + 1) * CH]
                e_pair = bass.AP(tensor=base.tensor, offset=base.offset,
                                 ap=[base.ap[0], [4 * CH, 2], [1, CH]])
                base2 = ea_t[:, b0 * CH:(b0 + 1) * CH]
                ea_pair = bass.AP(tensor=base2.tensor, offset=base2.offset,
                                  ap=[base2.ap[0], [4 * CH, 2], [1, CH]])
                a_sl = a_t[:, t * CH:(t + 1) * CH]
                a_bcast = bass.AP(tensor=a_sl.tensor, offset=a_sl.offset,
                                  ap=[a_sl.ap[0], [0, HPC], a_sl.ap[1]])
                nc.vector.tensor_mul(ea_pair, e_pair, a_bcast)
            if pending is not None:
                emit_phase2(*pending)
            pending = (e_t, ea_t, ch)
        emit_phase2(*pending)

    nc.finalize()
    return nc


def kernel(h, A, Wq, bq, Wk, bk, Wv, bv):
    global LAST_RESULTS
    from concourse.bass_utils import run_bass_kernel_spmd

    h = np.asarray(h, np.float32)
    A = np.asarray(A, np.float32)
    Wq = np.asarray(Wq, np.float32)
    Wk = np.asarray(Wk, np.float32)
    Wv = np.asarray(Wv, np.float32)
    bq = np.asarray(bq, np.float32)
    bk = np.asarray(bk, np.float32)
    bv = np.asarray(bv, np.float32)

    hT = np.ascontiguousarray(h.transpose(0, 2, 1)).astype(ml_dtypes.bfloat16)
    Ab = np.ascontiguousarray(A.astype(ml_dtypes.bfloat16))  # [B, N, N]
    sc = np.float32(1.0 / math.sqrt(D))

    in_maps = []
    for c in range(NCORES):
        b = c // CORES_PER_B
        h0 = HPC * (c % CORES_PER_B)
        sl = slice(h0 * D, (h0 + HPC) * D)
        bk2 = bk[sl].reshape(HPC, D)                    # [head, d]
        bk4 = np.empty((128, HPC), np.float32)
        for hh in range(HPC):
            bk4[:, hh] = np.tile(bk2[hh], 4)            # strip-replicated
        in_maps.append({
            "hT": hT[b],
            "Ab": Ab[b],
            "wq": np.ascontiguousarray(Wq[:, sl]).astype(ml_dtypes.bfloat16),
            "wk": np.ascontiguousarray(Wk[:, sl]).astype(ml_dtypes.bfloat16),
            "wv": np.ascontiguousarray(Wv[:, sl]).astype(ml_dtypes.bfloat16),
            "bq": np.ascontiguousarray((bq[sl] * sc).reshape(-1, 1)),
            "bk4": bk4,
            "bvb": np.ascontiguousarray(np.tile(bv[sl][None, :], (128, 1))),
        })

    nc = _build_bass()
    res = run_bass_kernel_spmd(
        nc, in_maps, core_ids=list(range(NCORES)),
        trace=os.environ.get("BASS_TRACE", "0") == "1",
    )
    LAST_RESULTS = res

    out = np.empty((B, HEADS, N, D), np.float32)
    for c in range(NCORES):
        b = c // CORES_PER_B
        h0 = HPC * (c % CORES_PER_B)
        oo = res.results[c]["o"]                  # [128, N] f32
        for hh in range(HPC):
            num = oo[hh * D:(hh + 1) * D, :]      # [32, N] unnormalized out^T
            den = oo[64 + 32 * hh, :]             # [N]
            out[b, h0 + hh] = (num / den[None, :]).T
    return out



# revision 6
# speedup vs baseline: 1.0066x; 1.0066x over previous
"""Adjacency-aware multi-head attention on 8 trn2 NeuronCores.

Math (per b, head k):
  Q = h[b] @ Wq[:, k] + bq[k]           [N, D]
  S[i, j] = (Q_i . K_j) / sqrt(D)
  P[j, i] = exp(S[i, j]) / sum_j exp(S[i, j])      (softmax over keys j)
  out[i, d] = sum_j P[j, i] * A[b, j, i] * V[j, d]

The K bias cancels: it adds g[i] = Q_i . bk to every score of query i,
and softmax over j is invariant to per-i shifts -> bk is dropped.

Sharding: 16 (b, head) pairs over 8 cores, 2 heads of the SAME b per core so
the A[b] stream is shared by both heads.

Device dataflow ([j, i] layout so A needs no transpose).  The steady state is
paced by the ACT engine (exp of all scores); everything else hides under it:
  - PE warm-up: dummy matmuls + first half of the V projection run during the
    input-DMA wait so the HAM clock gate reaches 2.4 GHz before the main loop.
  - Strip mapping r = 2*head + (t%2): j-tile t of head hh computes on PE row
    strip r.  Q^T is written strip-replicated directly by the Q projection
    (host passes Wq with columns [h0|h0|h1|h1]), no SBUF->SBUF copies.
  - K^T packed per strip, K projection runs as 8 matmuls of N=1024.
  - exp on ACT: PSUM -> SBUF bf16, 3 PSUM banks per op.
  - EA = E * A on DVE: one 4D-AP tensor_tensor per j-tile pair (2x bf16 mode).
  - phase 2 (eager, lagged ~2 exp-groups): per j-tile 4 streams into one PSUM
    tile via column tiling: out_h0 (cols 0-31), out_h1 (32-63), denom_h0
    (col 64), denom_h1 (col 96).
Device returns [128, N]: rows 0-31 outT_h0, 32-63 outT_h1, rows 64/96 the
softmax denominators.  Host does out = (outT / denom)^T plus the gather.
"""

import math
import os

import numpy as np
import ml_dtypes

B, N, IN_DIM = 2, 2048, 256
HEADS, D = 8, 32
NCORES = 8
HPC = 2              # heads per core
NJ = N // 128        # 16 j-tiles
NCH = 4              # i-chunks
CH = N // NCH        # 512
CORES_PER_B = NCORES // B
GRP = 3              # S psum banks per exp op
NBLK = NJ * HPC      # 32 S blocks per chunk
NGRP = (NBLK + GRP - 1) // GRP   # 11 exp groups per chunk
NPAIR = NJ // 2      # 8 j-tile pairs

LAST_RESULTS = None  # BassKernelResults of the most recent kernel() call


def _build_bass():
    import concourse.bass as bass
    import concourse.mybir as mybir
    import concourse.tile as tile
    from concourse import bacc

    f32 = mybir.dt.float32
    bf16 = mybir.dt.bfloat16
    AF = mybir.ActivationFunctionType

    nc = bacc.Bacc("TRN2", target_bir_lowering=False, debug=False,
                   num_devices=NCORES)

    hT = nc.dram_tensor("hT", [IN_DIM, N], bf16, kind="ExternalInput").ap()
    Ab = nc.dram_tensor("Ab", [N, N], bf16, kind="ExternalInput").ap()
    wq = nc.dram_tensor("wq", [IN_DIM, 4 * D], bf16, kind="ExternalInput").ap()
    wk = nc.dram_tensor("wk", [IN_DIM, HPC * D], bf16, kind="ExternalInput").ap()
    wv = nc.dram_tensor("wv", [IN_DIM, HPC * D], bf16, kind="ExternalInput").ap()
    bq4 = nc.dram_tensor("bq4", [128, 1], f32, kind="ExternalInput").ap()
    bvb = nc.dram_tensor("bvb", [128, HPC * D], f32, kind="ExternalInput").ap()
    o = nc.dram_tensor("o", [128, N], f32, kind="ExternalOutput").ap()

    SC = 1.0 / math.sqrt(D)

    def bcast_free(ap_col, n):
        return bass.AP(tensor=ap_col.tensor, offset=ap_col.offset,
                       ap=[ap_col.ap[0], [0, n]])

    # block index for (head hh, j-tile t): strip r = 2*hh + t%2
    def blk(hh, t):
        return 4 * (t // 2) + 2 * hh + (t % 2)

    with (
        tile.TileContext(nc) as tc,
        tc.tile_pool(name="const", bufs=1) as cpool,
        tc.tile_pool(name="ps", bufs=2, space="PSUM") as pspool,
        tc.tile_pool(name="pod", bufs=2, space="PSUM") as podpool,
        tc.tile_pool(name="apool", bufs=2) as apool,
        tc.tile_pool(name="epool", bufs=2) as epool,
        tc.tile_pool(name="eapool", bufs=2) as eapool,
        tc.tile_pool(name="opool", bufs=2) as opool,
    ):
        # ---- constants / inputs into SBUF
        scratch = cpool.tile([128, CH], bf16, tag="scratch")
        nc.vector.memset(scratch, 0.0)
        ones_sb = cpool.tile([128, 1], bf16, tag="ones")
        nc.vector.memset(ones_sb, 1.0)

        hT_sb = cpool.tile([128, 2, N], bf16, tag="hT")
        hT3 = hT.rearrange("(s p) n -> p s n", p=128)
        for q in range(4):
            nc.sync.dma_start(hT_sb[:, :, q * CH:(q + 1) * CH],
                              hT3[:, :, q * CH:(q + 1) * CH])
        w_sb = {}
        for name, ap, m in (("q", wq, 4 * D), ("k", wk, HPC * D),
                            ("v", wv, HPC * D)):
            t = cpool.tile([128, 2, m], bf16, tag=f"w{name}")
            nc.sync.dma_start(t, ap.rearrange("(s p) m -> p s m", p=128))
            w_sb[name] = t
        bq4_sb = cpool.tile([128, 1], f32, tag="bq4")
        nc.sync.dma_start(bq4_sb, bq4)
        bvb_sb = cpool.tile([128, HPC * D], f32, tag="bvb")
        nc.sync.dma_start(bvb_sb, bvb)

        qt4 = cpool.tile([128, N], bf16, tag="qt4")      # strips [h0|h0|h1|h1]
        kt4 = cpool.tile([128, NJ // 2 * 128], bf16, tag="kt4")
        Vt = cpool.tile([128, NJ * HPC * D], bf16, tag="vt")   # col = t*64+d

        A3 = Ab.rearrange("(t p) i -> p t i", p=128)
        a_tiles = [None] * NCH

        def emit_a_dma(ch):
            a_t = apool.tile([128, NJ, CH], bf16, tag="a")
            nc.sync.dma_start(a_t, A3[:, :, ch * CH:(ch + 1) * CH])
            a_tiles[ch] = a_t

        emit_a_dma(0)

        # ---- PE warm-up: dummy matmuls while input DMAs are in flight
        for _ in range(3):
            jt = podpool.tile([128, CH], f32, tag="od", name="junk")
            nc.tensor.matmul(jt, lhsT=scratch[:, 0:128], rhs=scratch,
                             start=True, stop=True)

        # ---- V projection, first half (tiles 0-7) during the DMA wait
        def emit_vproj_mms(vps, base, t0, cnt):
            for t in range(t0, t0 + cnt):
                for s in range(2):
                    nc.tensor.matmul(
                        vps[:, (t - base) * HPC * D:(t - base + 1) * HPC * D],
                        lhsT=hT_sb[:, s, t * 128:(t + 1) * 128],
                        rhs=w_sb["v"][:, s, :],
                        start=(s == 0), stop=(s == 1),
                    )

        def emit_vproj_add(vps, t0):
            base = Vt[:, t0 * HPC * D:(t0 + 8) * HPC * D]
            out_ap = bass.AP(tensor=base.tensor, offset=base.offset,
                             ap=[base.ap[0], [HPC * D, 8], [1, HPC * D]])
            in_ap = bass.AP(tensor=vps.tensor, offset=vps.offset,
                            ap=[vps.ap[0], [HPC * D, 8], [1, HPC * D]])
            b_ap = bass.AP(tensor=bvb_sb.tensor, offset=bvb_sb.offset,
                           ap=[bvb_sb.ap[0], [0, 8], [1, HPC * D]])
            nc.vector.tensor_add(out_ap, in_ap, b_ap)

        vps0 = podpool.tile([128, CH], f32, tag="od", name="vps0")
        emit_vproj_mms(vps0, 0, 0, 8)
        emit_vproj_add(vps0, 0)

        # ---- K projection into packed strip layout, N=1024 matmuls.
        #      strip r holds K^T of head r//2 for tiles t = 2*q2 + r%2.
        kps = pspool.tile([128, NJ // 2 * 128], f32, tag="ps", name="kps")
        for r in range(4):
            for c in range(2):
                for s in range(2):
                    base = hT_sb[:, s, (r % 2) * 128 + c * 1024:
                                 (r % 2) * 128 + c * 1024 + 128]
                    rhs = bass.AP(tensor=base.tensor, offset=base.offset,
                                  ap=[base.ap[0], [256, 4], [1, 128]])
                    nc.tensor.matmul(
                        kps[32 * r:32 * r + 32, c * CH:(c + 1) * CH],
                        lhsT=w_sb["k"][:, s, (r // 2) * D:(r // 2 + 1) * D],
                        rhs=rhs,
                        start=(s == 0), stop=(s == 1),
                        tile_position=(0, 32 * r),
                    )
        nc.vector.tensor_copy(kt4, kps)   # bk dropped (cancels in softmax)

        # ---- Q projection (+bias, scaled 1/sqrt(D)), strip-replicated
        #      directly: wq columns are [h0|h0|h1|h1] (host-packed).
        def emit_qproj(quarter):
            sl = slice(quarter * CH, (quarter + 1) * CH)
            qps = pspool.tile([128, CH], f32, tag="ps", name="qps")
            for s in range(2):
                nc.tensor.matmul(qps, lhsT=w_sb["q"][:, s, :],
                                 rhs=hT_sb[:, s, sl],
                                 start=(s == 0), stop=(s == 1))
            nc.vector.scalar_tensor_tensor(
                qt4[:, sl], qps, SC, bcast_free(bq4_sb, CH),
                op0=mybir.AluOpType.mult, op1=mybir.AluOpType.add,
            )

        emit_qproj(0)

        # ---- main loop helpers
        def emit_amult(e_t, ea_t, a_t, p):
            eb = e_t[:, 4 * p * CH:(4 * p + 1) * CH]
            e_ap = bass.AP(tensor=eb.tensor, offset=eb.offset,
                           ap=[eb.ap[0], [2 * CH, 2], [CH, 2], [1, CH]])
            eab = ea_t[:, 4 * p * CH:(4 * p + 1) * CH]
            ea_ap = bass.AP(tensor=eab.tensor, offset=eab.offset,
                            ap=[eab.ap[0], [2 * CH, 2], [CH, 2], [1, CH]])
            ab = a_t[:, 2 * p, :]
            a_ap = bass.AP(tensor=ab.tensor, offset=ab.offset,
                           ap=[ab.ap[0], [0, 2], [CH, 2], [1, CH]])
            nc.vector.tensor_mul(ea_ap, e_ap, a_ap)

        def emit_ph2_pair(od, e_t, ea_t, p):
            for tp in range(2):
                t = 2 * p + tp
                first, last = (t == 0), (t == NJ - 1)
                for hh in range(HPC):
                    bsl = slice(blk(hh, t) * CH, (blk(hh, t) + 1) * CH)
                    nc.tensor.matmul(
                        od[32 * hh:32 * hh + 32, :],
                        lhsT=Vt[:, t * 64 + 32 * hh:t * 64 + 32 * hh + 32],
                        rhs=ea_t[:, bsl],
                        start=first, stop=last, tile_position=(0, 32 * hh),
                    )
                for hh in range(HPC):
                    bsl = slice(blk(hh, t) * CH, (blk(hh, t) + 1) * CH)
                    nc.tensor.matmul(
                        od[64 + 32 * hh:65 + 32 * hh, :],
                        lhsT=ones_sb, rhs=e_t[:, bsl],
                        start=first, stop=last,
                        tile_position=(0, 64 + 32 * hh),
                    )

        def emit_out(od, ch):
            o_sb = opool.tile([128, CH], f32, tag="o")
            nc.vector.tensor_copy(o_sb, od)
            nc.sync.dma_start(o[:, ch * CH:(ch + 1) * CH], o_sb)

        # pair p's exp group: the group containing block 4p+3
        pair_ready = [(4 * p + 3) // GRP for p in range(NPAIR)]

        carry = None   # (od, e_t, ea_t, ch) pairs 6,7 + flush of previous chunk
        for ch in range(NCH):
            if ch + 1 < NCH:
                emit_a_dma(ch + 1)
            a_t = a_tiles[ch]
            e_t = epool.tile([128, NBLK * CH], bf16, tag="e")
            ea_t = eapool.tile([128, NBLK * CH], bf16, tag="ea")
            od = None

            # inline insert schedule for this chunk
            amult_at = {}    # group -> list of pairs to multiply
            ph2_at = {}      # group -> list of pairs for phase2 (lag 2 groups)
            for p in range(NPAIR):
                amult_at.setdefault(pair_ready[p], []).append(p)
                g2 = pair_ready[p] + 2
                if g2 < NGRP:
                    ph2_at.setdefault(g2, []).append(p)

            for g in range(NGRP):
                b_lo = g * GRP
                b_hi = min(b_lo + GRP, NBLK)
                ps = pspool.tile([128, GRP * CH], f32, tag="ps", name="sps")
                for b in range(b_lo, b_hi):
                    q2, r = b // 4, b % 4
                    nc.tensor.matmul(
                        ps[:, (b - b_lo) * CH:(b - b_lo + 1) * CH],
                        lhsT=kt4[32 * r:32 * r + 32, q2 * 128:(q2 + 1) * 128],
                        rhs=qt4[32 * r:32 * r + 32, ch * CH:(ch + 1) * CH],
                        start=True, stop=True,
                        tile_position=(32 * r, 0),
                    )
                nc.scalar.activation(
                    e_t[:, b_lo * CH:b_hi * CH], ps[:, :(b_hi - b_lo) * CH],
                    AF.Exp)

                if g == 0 and carry is not None:
                    cod, ce, cea, cch = carry
                    for p in (6, 7):
                        emit_ph2_pair(cod, ce, cea, p)
                    emit_out(cod, cch)
                    carry = None

                for p in amult_at.get(g, ()):
                    emit_amult(e_t, ea_t, a_t, p)
                for p in ph2_at.get(g, ()):
                    if od is None:
                        od = podpool.tile([128, CH], f32, tag="od")
                    emit_ph2_pair(od, e_t, ea_t, p)

                if ch == 0:
                    if g == 0:
                        vps1 = podpool.tile([128, CH], f32, tag="od",
                                            name="vps1")
                        emit_vproj_mms(vps1, 8, 8, 4)
                    elif g == 1:
                        emit_vproj_mms(vps1, 8, 12, 4)
                        emit_vproj_add(vps1, 8)
                    elif g in (2, 5, 8):
                        emit_qproj({2: 1, 5: 2, 8: 3}[g])

            carry = (od, e_t, ea_t, ch)

        cod, ce, cea, cch = carry
        for p in (6, 7):
            emit_ph2_pair(cod, ce, cea, p)
        emit_out(cod, cch)

    nc.finalize()
    return nc


def kernel(h, A, Wq, bq, Wk, bk, Wv, bv):
    global LAST_RESULTS
    from concourse.bass_utils import run_bass_kernel_spmd

    h = np.asarray(h, np.float32)
    A = np.asarray(A, np.float32)
    Wq = np.asarray(Wq, np.float32)
    Wk = np.asarray(Wk, np.float32)
    Wv = np.asarray(Wv, np.float32)
    bq = np.asarray(bq, np.float32)
    bv = np.asarray(bv, np.float32)

    hT = np.ascontiguousarray(h.transpose(0, 2, 1)).astype(ml_dtypes.bfloat16)
    Ab = np.ascontiguousarray(A.astype(ml_dtypes.bfloat16))  # [B, N, N]
    sc = np.float32(1.0 / math.sqrt(D))

    in_maps = []
    for c in range(NCORES):
        b = c // CORES_PER_B
        h0 = HPC * (c % CORES_PER_B)
        sl = slice(h0 * D, (h0 + HPC) * D)
        wq_h = [Wq[:, (h0 + k) * D:(h0 + k + 1) * D] for k in range(HPC)]
        wq_rep = np.concatenate([wq_h[0], wq_h[0], wq_h[1], wq_h[1]], axis=1)
        bq_h = [bq[(h0 + k) * D:(h0 + k + 1) * D] for k in range(HPC)]
        bq4 = np.concatenate([bq_h[0], bq_h[0], bq_h[1], bq_h[1]]) * sc
        in_maps.append({
            "hT": hT[b],
            "Ab": Ab[b],
            "wq": np.ascontiguousarray(wq_rep).astype(ml_dtypes.bfloat16),
            "wk": np.ascontiguousarray(Wk[:, sl]).astype(ml_dtypes.bfloat16),
            "wv": np.ascontiguousarray(Wv[:, sl]).astype(ml_dtypes.bfloat16),
            "bq4": np.ascontiguousarray(bq4.reshape(128, 1)),
            "bvb": np.ascontiguousarray(np.tile(bv[sl][None, :], (128, 1))),
        })

    nc = _build_bass()
    res = run_bass_kernel_spmd(
        nc, in_maps, core_ids=list(range(NCORES)),
        trace=os.environ.get("BASS_TRACE", "0") == "1",
    )
    LAST_RESULTS = res

    out = np.empty((B, HEADS, N, D), np.float32)
    for c in range(NCORES):
        b = c // CORES_PER_B
        h0 = HPC * (c % CORES_PER_B)
        oo = res.results[c]["o"]                  # [128, N] f32
        for hh in range(HPC):
            num = oo[hh * D:(hh + 1) * D, :]      # [32, N] unnormalized out^T
            den = oo[64 + 32 * hh, :]             # [N]
            out[b, h0 + hh] = (num / den[None, :]).T
    return out


# revision 7
# speedup vs baseline: 1.0489x; 1.0421x over previous
"""Adjacency-aware multi-head attention on 8 trn2 NeuronCores.

Math (per b, head k):
  Q = h[b] @ Wq[:, k] + bq[k]           [N, D]
  S[i, j] = (Q_i . K_j) / sqrt(D)
  P[j, i] = exp(S[i, j]) / sum_j exp(S[i, j])      (softmax over keys j)
  out[i, d] = sum_j P[j, i] * A[b, j, i] * V[j, d]

The K bias cancels: it adds g[i] = Q_i . bk to every score of query i,
and softmax over j is invariant to per-i shifts -> bk is dropped.

Sharding: 16 (b, head) pairs over 8 cores, 2 heads of the SAME b per core so
the A[b] stream is shared by both heads.

Device dataflow ([j, i] layout so A needs no transpose).  The steady state is
paced by the ACT engine (exp of all scores); everything else hides under it:
  - PE warm-up: dummy matmuls + first half of the V projection run during the
    input-DMA wait so the HAM clock gate reaches 2.4 GHz before the main loop.
  - Strip mapping r = 2*head + (t%2): j-tile t of head hh computes on PE row
    strip r.  Q^T is written strip-replicated directly by the Q projection
    (host passes Wq with columns [h0|h0|h1|h1]), no SBUF->SBUF copies.
  - K^T packed per strip, K projection runs as 8 matmuls of N=1024.
  - exp on ACT: PSUM -> SBUF bf16, 3 PSUM banks per op.
  - EA = E * A on DVE: one 4D-AP tensor_tensor per j-tile pair (2x bf16 mode).
  - phase 2 (eager, lagged ~2 exp-groups): per j-tile 4 streams into one PSUM
    tile via column tiling: out_h0 (cols 0-31), out_h1 (32-63), denom_h0
    (col 64), denom_h1 (col 96).
Device returns [128, N]: rows 0-31 outT_h0, 32-63 outT_h1, rows 64/96 the
softmax denominators.  Host does out = (outT / denom)^T plus the gather.
"""

import math
import os

import numpy as np
import ml_dtypes

B, N, IN_DIM = 2, 2048, 256
HEADS, D = 8, 32
NCORES = 8
HPC = 2              # heads per core
NJ = N // 128        # 16 j-tiles
NCH = 4              # i-chunks
CH = N // NCH        # 512
CORES_PER_B = NCORES // B
GRP = 3              # S psum banks per exp op
NBLK = NJ * HPC      # 32 S blocks per chunk
NGRP = (NBLK + GRP - 1) // GRP   # 11 exp groups per chunk
NPAIR = NJ // 2      # 8 j-tile pairs

LAST_RESULTS = None  # BassKernelResults of the most recent kernel() call


def _build_bass():
    import concourse.bass as bass
    import concourse.mybir as mybir
    import concourse.tile as tile
    from concourse import bacc

    f32 = mybir.dt.float32
    bf16 = mybir.dt.bfloat16
    AF = mybir.ActivationFunctionType

    nc = bacc.Bacc("TRN2", target_bir_lowering=False, debug=False,
                   num_devices=NCORES)

    hT = nc.dram_tensor("hT", [IN_DIM, N], bf16, kind="ExternalInput").ap()
    Ab = nc.dram_tensor("Ab", [N, N], bf16, kind="ExternalInput").ap()
    wq = nc.dram_tensor("wq", [IN_DIM, 4 * D], bf16, kind="ExternalInput").ap()
    wk = nc.dram_tensor("wk", [IN_DIM, HPC * D], bf16, kind="ExternalInput").ap()
    wv = nc.dram_tensor("wv", [IN_DIM, HPC * D], bf16, kind="ExternalInput").ap()
    bq4 = nc.dram_tensor("bq4", [128, 1], f32, kind="ExternalInput").ap()
    bvb = nc.dram_tensor("bvb", [128, HPC * D], f32, kind="ExternalInput").ap()
    o = nc.dram_tensor("o", [128, N], f32, kind="ExternalOutput").ap()

    SC = 1.0 / math.sqrt(D)

    def bcast_free(ap_col, n):
        return bass.AP(tensor=ap_col.tensor, offset=ap_col.offset,
                       ap=[ap_col.ap[0], [0, n]])

    # block index for (head hh, j-tile t): strip r = 2*hh + t%2
    def blk(hh, t):
        return 4 * (t // 2) + 2 * hh + (t % 2)

    with (
        tile.TileContext(nc) as tc,
        tc.tile_pool(name="const", bufs=1) as cpool,
        tc.tile_pool(name="ps", bufs=2, space="PSUM") as pspool,
        tc.tile_pool(name="pod", bufs=2, space="PSUM") as podpool,
        tc.tile_pool(name="apool", bufs=2) as apool,
        tc.tile_pool(name="epool", bufs=2) as epool,
        tc.tile_pool(name="eapool", bufs=2) as eapool,
        tc.tile_pool(name="opool", bufs=2) as opool,
    ):
        # ---- constants / inputs into SBUF
        scratch = cpool.tile([128, CH], bf16, tag="scratch")
        nc.vector.memset(scratch, 0.0)
        ones_sb = cpool.tile([128, 1], bf16, tag="ones")
        nc.vector.memset(ones_sb, 1.0)

        hT_sb = cpool.tile([128, 2, N], bf16, tag="hT")
        hT3 = hT.rearrange("(s p) n -> p s n", p=128)
        for q in range(4):
            nc.sync.dma_start(hT_sb[:, :, q * CH:(q + 1) * CH],
                              hT3[:, :, q * CH:(q + 1) * CH])
        w_sb = {}
        for name, ap, m in (("q", wq, 4 * D), ("k", wk, HPC * D),
                            ("v", wv, HPC * D)):
            t = cpool.tile([128, 2, m], bf16, tag=f"w{name}")
            nc.sync.dma_start(t, ap.rearrange("(s p) m -> p s m", p=128))
            w_sb[name] = t
        bq4_sb = cpool.tile([128, 1], f32, tag="bq4")
        nc.sync.dma_start(bq4_sb, bq4)
        bvb_sb = cpool.tile([128, HPC * D], f32, tag="bvb")
        nc.sync.dma_start(bvb_sb, bvb)

        qt4 = cpool.tile([128, N], bf16, tag="qt4")      # strips [h0|h0|h1|h1]
        kt4 = cpool.tile([128, NJ // 2 * 128], bf16, tag="kt4")
        Vt = cpool.tile([128, NJ * HPC * D], bf16, tag="vt")   # col = t*64+d

        A3 = Ab.rearrange("(t p) i -> p t i", p=128)
        a_tiles = [None] * NCH

        def emit_a_dma(ch):
            a_t = apool.tile([128, NJ, CH], bf16, tag="a")
            nc.sync.dma_start(a_t, A3[:, :, ch * CH:(ch + 1) * CH])
            a_tiles[ch] = a_t

        emit_a_dma(0)

        # ---- PE warm-up: dummy matmuls while input DMAs are in flight
        for _ in range(3):
            jt = podpool.tile([128, CH], f32, tag="od", name="junk")
            nc.tensor.matmul(jt, lhsT=scratch[:, 0:128], rhs=scratch,
                             start=True, stop=True)

        # ---- V projection, first half (tiles 0-7) during the DMA wait
        def emit_vproj_mms(vps, base, t0, cnt):
            for t in range(t0, t0 + cnt):
                for s in range(2):
                    nc.tensor.matmul(
                        vps[:, (t - base) * HPC * D:(t - base + 1) * HPC * D],
                        lhsT=hT_sb[:, s, t * 128:(t + 1) * 128],
                        rhs=w_sb["v"][:, s, :],
                        start=(s == 0), stop=(s == 1),
                    )

        def emit_vproj_add(vps, t0):
            base = Vt[:, t0 * HPC * D:(t0 + 8) * HPC * D]
            out_ap = bass.AP(tensor=base.tensor, offset=base.offset,
                             ap=[base.ap[0], [HPC * D, 8], [1, HPC * D]])
            in_ap = bass.AP(tensor=vps.tensor, offset=vps.offset,
                            ap=[vps.ap[0], [HPC * D, 8], [1, HPC * D]])
            b_ap = bass.AP(tensor=bvb_sb.tensor, offset=bvb_sb.offset,
                           ap=[bvb_sb.ap[0], [0, 8], [1, HPC * D]])
            nc.vector.tensor_add(out_ap, in_ap, b_ap)

        vps0 = podpool.tile([128, CH], f32, tag="od", name="vps0")
        emit_vproj_mms(vps0, 0, 0, 8)
        emit_vproj_add(vps0, 0)

        # ---- K projection into packed strip layout, N=1024 matmuls.
        #      strip r holds K^T of head r//2 for tiles t = 2*q2 + r%2.
        kps = pspool.tile([128, NJ // 2 * 128], f32, tag="ps", name="kps")
        for r in range(4):
            for c in range(2):
                for s in range(2):
                    base = hT_sb[:, s, (r % 2) * 128 + c * 1024:
                                 (r % 2) * 128 + c * 1024 + 128]
                    rhs = bass.AP(tensor=base.tensor, offset=base.offset,
                                  ap=[base.ap[0], [256, 4], [1, 128]])
                    nc.tensor.matmul(
                        kps[32 * r:32 * r + 32, c * CH:(c + 1) * CH],
                        lhsT=w_sb["k"][:, s, (r // 2) * D:(r // 2 + 1) * D],
                        rhs=rhs,
                        start=(s == 0), stop=(s == 1),
                        tile_position=(0, 32 * r),
                    )
        nc.vector.tensor_copy(kt4, kps)   # bk dropped (cancels in softmax)

        # ---- Q projection (+bias, scaled 1/sqrt(D)), strip-replicated
        #      directly: wq columns are [h0|h0|h1|h1] (host-packed).
        def emit_qproj(quarter):
            sl = slice(quarter * CH, (quarter + 1) * CH)
            qps = pspool.tile([128, CH], f32, tag="ps", name="qps")
            for s in range(2):
                nc.tensor.matmul(qps, lhsT=w_sb["q"][:, s, :],
                                 rhs=hT_sb[:, s, sl],
                                 start=(s == 0), stop=(s == 1))
            nc.vector.scalar_tensor_tensor(
                qt4[:, sl], qps, SC, bcast_free(bq4_sb, CH),
                op0=mybir.AluOpType.mult, op1=mybir.AluOpType.add,
            )

        emit_qproj(0)

        # ---- main loop helpers
        def emit_amult(e_t, ea_t, a_t, p, hh):
            # blocks 4p+2hh, 4p+2hh+1 = head hh, j-tiles 2p, 2p+1: contiguous
            b0 = (4 * p + 2 * hh) * CH
            eb = e_t[:, b0:b0 + CH]
            e_ap = bass.AP(tensor=eb.tensor, offset=eb.offset,
                           ap=[eb.ap[0], [CH, 2], [1, CH]])
            eab = ea_t[:, b0:b0 + CH]
            ea_ap = bass.AP(tensor=eab.tensor, offset=eab.offset,
                            ap=[eab.ap[0], [CH, 2], [1, CH]])
            nc.vector.tensor_mul(ea_ap, e_ap, a_t[:, 2 * p:2 * p + 2, :])

        def emit_ph2_quad(od, e_t, ea_t, p, hh):
            for tp in range(2):
                t = 2 * p + tp
                first, last = (t == 0), (t == NJ - 1)
                bsl = slice(blk(hh, t) * CH, (blk(hh, t) + 1) * CH)
                nc.tensor.matmul(
                    od[32 * hh:32 * hh + 32, :],
                    lhsT=Vt[:, t * 64 + 32 * hh:t * 64 + 32 * hh + 32],
                    rhs=ea_t[:, bsl],
                    start=first, stop=last, tile_position=(0, 32 * hh),
                )
                nc.tensor.matmul(
                    od[64 + 32 * hh:65 + 32 * hh, :],
                    lhsT=ones_sb, rhs=e_t[:, bsl],
                    start=first, stop=last,
                    tile_position=(0, 64 + 32 * hh),
                )

        def emit_out(od, ch):
            o_sb = opool.tile([128, CH], f32, tag="o")
            nc.vector.tensor_copy(o_sb, od)
            for q in range(4):
                nc.sync.dma_start(
                    o[:, ch * CH + q * 128:ch * CH + (q + 1) * 128],
                    o_sb[:, q * 128:(q + 1) * 128])

        carry = None   # (od, e_t, ea_t, ch, quads) spill of previous chunk
        for ch in range(NCH):
            if ch + 1 < NCH:
                emit_a_dma(ch + 1)
            a_t = a_tiles[ch]
            e_t = epool.tile([128, NBLK * CH], bf16, tag="e")
            ea_t = eapool.tile([128, NBLK * CH], bf16, tag="ea")
            od = None

            # inline insert schedule: (p, hh) half-pair ready after the exp
            # group covering block 4p+2hh+1; phase2 lags LAG groups behind.
            LAG = 1 if ch == NCH - 1 else 2
            amult_at = {}
            ph2_at = {}
            spill = []
            for p in range(NPAIR):
                for hh in range(HPC):
                    rg = (4 * p + 2 * hh + 1) // GRP
                    amult_at.setdefault(rg, []).append((p, hh))
                    if rg + LAG < NGRP:
                        ph2_at.setdefault(rg + LAG, []).append((p, hh))
                    else:
                        spill.append((p, hh))

            for g in range(NGRP):
                b_lo = g * GRP
                b_hi = min(b_lo + GRP, NBLK)
                ps = pspool.tile([128, GRP * CH], f32, tag="ps", name="sps")
                for b in range(b_lo, b_hi):
                    q2, r = b // 4, b % 4
                    nc.tensor.matmul(
                        ps[:, (b - b_lo) * CH:(b - b_lo + 1) * CH],
                        lhsT=kt4[32 * r:32 * r + 32, q2 * 128:(q2 + 1) * 128],
                        rhs=qt4[32 * r:32 * r + 32, ch * CH:(ch + 1) * CH],
                        start=True, stop=True,
                        tile_position=(32 * r, 0),
                    )
                nc.scalar.activation(
                    e_t[:, b_lo * CH:b_hi * CH], ps[:, :(b_hi - b_lo) * CH],
                    AF.Exp)

                for p, hh in amult_at.get(g, ()):
                    emit_amult(e_t, ea_t, a_t, p, hh)

                # drain previous chunk's spilled phase2 quads, 1-2 per group
                if carry is not None and g <= 2:
                    cod, ce, cea, cch, cquads = carry
                    take = cquads[:2] if g < 2 else cquads
                    for p, hh in take:
                        emit_ph2_quad(cod, ce, cea, p, hh)
                    cquads = cquads[len(take):]
                    if not cquads:
                        emit_out(cod, cch)
                        carry = None
                    else:
                        carry = (cod, ce, cea, cch, cquads)

                for p, hh in ph2_at.get(g, ()):
                    if od is None:
                        od = podpool.tile([128, CH], f32, tag="od")
                    emit_ph2_quad(od, e_t, ea_t, p, hh)

                if ch == 0:
                    if g == 0:
                        vps1 = podpool.tile([128, CH], f32, tag="od",
                                            name="vps1")
                        emit_vproj_mms(vps1, 8, 8, 4)
                    elif g == 1:
                        emit_vproj_mms(vps1, 8, 12, 4)
                        emit_vproj_add(vps1, 8)
                    elif g in (2, 5, 8):
                        emit_qproj({2: 1, 5: 2, 8: 3}[g])

            carry = (od, e_t, ea_t, ch, spill)

        cod, ce, cea, cch, cquads = carry
        for p, hh in cquads:
            emit_ph2_quad(cod, ce, cea, p, hh)
        emit_out(cod, cch)

    nc.finalize()
    return nc


def kernel(h, A, Wq, bq, Wk, bk, Wv, bv):
    global LAST_RESULTS
    from concourse.bass_utils import run_bass_kernel_spmd

    h = np.asarray(h, np.float32)
    A = np.asarray(A, np.float32)
    Wq = np.asarray(Wq, np.float32)
    Wk = np.asarray(Wk, np.float32)
    Wv = np.asarray(Wv, np.float32)
    bq = np.asarray(bq, np.float32)
    bv = np.asarray(bv, np.float32)

    hT = np.ascontiguousarray(h.transpose(0, 2, 1)).astype(ml_dtypes.bfloat16)
    Ab = np.ascontiguousarray(A.astype(ml_dtypes.bfloat16))  # [B, N, N]
    sc = np.float32(1.0 / math.sqrt(D))

    in_maps = []
    for c in range(NCORES):
        b = c // CORES_PER_B
        h0 = HPC * (c % CORES_PER_B)
        sl = slice(h0 * D, (h0 + HPC) * D)
        wq_h = [Wq[:, (h0 + k) * D:(h0 + k + 1) * D] for k in range(HPC)]
        wq_rep = np.concatenate([wq_h[0], wq_h[0], wq_h[1], wq_h[1]], axis=1)
        bq_h = [bq[(h0 + k) * D:(h0 + k + 1) * D] for k in range(HPC)]
        bq4 = np.concatenate([bq_h[0], bq_h[0], bq_h[1], bq_h[1]]) * sc
        in_maps.append({
            "hT": hT[b],
            "Ab": Ab[b],
            "wq": np.ascontiguousarray(wq_rep).astype(ml_dtypes.bfloat16),
            "wk": np.ascontiguousarray(Wk[:, sl]).astype(ml_dtypes.bfloat16),
            "wv": np.ascontiguousarray(Wv[:, sl]).astype(ml_dtypes.bfloat16),
            "bq4": np.ascontiguousarray(bq4.reshape(128, 1)),
            "bvb": np.ascontiguousarray(np.tile(bv[sl][None, :], (128, 1))),
        })

    nc = _build_bass()
    res = run_bass_kernel_spmd(
        nc, in_maps, core_ids=list(range(NCORES)),
        trace=os.environ.get("BASS_TRACE", "0") == "1",
    )
    LAST_RESULTS = res

    out = np.empty((B, HEADS, N, D), np.float32)
    for c in range(NCORES):
        b = c // CORES_PER_B
        h0 = HPC * (c % CORES_PER_B)
        oo = res.results[c]["o"]                  # [128, N] f32
        for hh in range(HPC):
            num = oo[hh * D:(hh + 1) * D, :]      # [32, N] unnormalized out^T
            den = oo[64 + 32 * hh, :]             # [N]
            out[b, h0 + hh] = (num / den[None, :]).T
    return out
